# revision 48
# baseline (speedup 1.0000x reference)
"""Trainium2 Bass kernel for CausalGNNCore (gnn_message_passing).

Math (reference, B=128 d=64 H=128):
  h   = tanh(X[...,None] * enc_w + enc_b)                 (B,d,H)
  u   = h @ msg_w1[:H];  v = h @ msg_w1[H:]               (B,d,H)
  pre[b,i,j] = u[b,j] + v[b,i] + msg_b1                   (B,d,d,H)
  msg = relu(pre) @ msg_w2 + msg_b2
  agg[b,i] = sum_j A[j,i] * msg[b,i,j],  A = W*(1-I)
  z   = relu([h, agg] @ dec_w1 + dec_b1);  out = z @ dec_w2 + dec_b2

Key identity used on-device: the msg_w2 matmul and the A-weighted sum over j
are both linear, so they commute:
  agg[b,i] = (sum_j A[j,i] * relu(pre[b,i,j])) @ msg_w2 + (sum_j A[j,i]) * msg_b2
           =: R[b,i] @ msg_w2 + s_i * msg_b2
This removes the (B*d*d, H) @ (H, H) matmul entirely; the remaining hot work
is elementwise over the (B,d,d,H) volume: add (broadcasted u+v), relu,
multiply by A (broadcast over H), and a segmented reduce over j.

Per-core layout: H on partitions, (i, j) or (b, node) on the free dim.
Data-parallel over batch: core c handles batches [16c, 16c+16).

Folds done on host (O(weights) numpy only):
  A     = W * (1 - I); s = A.sum(0)
  Wf    = msg_w2 @ dec_w1[H:]        (R feeds the decoder directly)
  cvec  = dec_w1[H:].T @ msg_b2      (rank-1 bias term s_i * cvec[m])
"""

import os
import sys

sys.path.insert(0, "/opt/trn_rl_repo")

import numpy as np

import concourse.bacc as bacc
import concourse.bass as bass
import concourse.tile as tile
from concourse import mybir
from concourse.bass_utils import run_bass_kernel_spmd

B, D, H = 128, 64, 128
NCORES = 8
BL = B // NCORES  # local batches per core
COLS = BL * D     # 1024 free columns for (b, node) tensors

_CACHE = {}

F32 = mybir.dt.float32
F16 = mybir.dt.float16
# i-rows whose A-weighted j-reduction runs on DVE (mult + fold tree) instead
# of the DMA-transpose + PE path; balances the two engine pipelines.
I_DVE = int(os.environ.get("K_I_DVE", "16"))
# i-rows whose u+v+relu runs as one DVE broadcasted add (+ ACT slice relu);
# the remaining rows run as per-i fused add+relu ACT instructions.
S_ADD = int(os.environ.get("K_S_ADD", "64"))
WORK_BUFS = int(os.environ.get("K_WORK_BUFS", "2"))
# ablation switch for perf diagnosis: "full" | "noreduce" | "onlyreduce"
ABLATE = os.environ.get("K_ABLATE", "full")


def _build_bass(repeat=1):
    nc = bacc.Bacc("TRN2", target_bir_lowering=False, debug=False,
                   num_devices=NCORES)

    # --- DRAM I/O ---
    xf = nc.dram_tensor("xf", [1, COLS], F32, kind="ExternalInput")
    encw = nc.dram_tensor("encw", [H, 1], F32, kind="ExternalInput")
    encb = nc.dram_tensor("encb", [H, 1], F32, kind="ExternalInput")
    w1a = nc.dram_tensor("w1a", [H, H], F32, kind="ExternalInput")
    w1b = nc.dram_tensor("w1b", [H, H], F32, kind="ExternalInput")
    b1 = nc.dram_tensor("b1", [H, 1], F32, kind="ExternalInput")
    acol2 = nc.dram_tensor("acol2", [H, D], mybir.dt.float16,
                           kind="ExternalInput")
    abc = None
    if I_DVE > 0:
        abc = nc.dram_tensor("abc", [H, I_DVE, D], mybir.dt.float16,
                             kind="ExternalInput")
    dw1a = nc.dram_tensor("dw1a", [H, H], F32, kind="ExternalInput")
    wf = nc.dram_tensor("wf", [H, H], F32, kind="ExternalInput")
    cvec = nc.dram_tensor("cvec", [H, 1], F32, kind="ExternalInput")
    stile = nc.dram_tensor("stile", [1, COLS], F32, kind="ExternalInput")
    db1 = nc.dram_tensor("db1", [H, 1], F32, kind="ExternalInput")
    dw2 = nc.dram_tensor("dw2", [H, 1], F32, kind="ExternalInput")
    db2 = nc.dram_tensor("db2", [1, 1], F32, kind="ExternalInput")
    out = nc.dram_tensor("out", [1, COLS], F32, kind="ExternalOutput")

    with tile.TileContext(nc) as tc:
        with (
            tc.tile_pool(name="consts", bufs=1) as consts,
            tc.tile_pool(name="persist", bufs=1) as persist,
            tc.tile_pool(name="work", bufs=WORK_BUFS) as work,
            tc.tile_pool(name="psum", bufs=2, space="PSUM") as psum,
            tc.tile_pool(name="psum_r", bufs=min(WORK_BUFS, 2), space="PSUM") as psum_r,
            tc.tile_pool(name="psum_z", bufs=1, space="PSUM") as psum_z,
        ):
            # --- load constants ---
            encw_s = consts.tile([H, 1], F32)
            nc.gpsimd.dma_start(out=encw_s[:], in_=encw[:])
            encb_s = consts.tile([H, 1], F32)
            nc.gpsimd.dma_start(out=encb_s[:], in_=encb[:])
            w1a_s = consts.tile([H, H], F32)
            nc.gpsimd.dma_start(out=w1a_s[:], in_=w1a[:])
            w1b_s = consts.tile([H, H], F32)
            nc.gpsimd.dma_start(out=w1b_s[:], in_=w1b[:])
            b1_s = consts.tile([H, 1], F32)
            nc.gpsimd.dma_start(out=b1_s[:], in_=b1[:])
            acol2_s = consts.tile([H, D], F16)
            nc.gpsimd.dma_start(out=acol2_s[:], in_=acol2[:])
            abc_s = None
            if I_DVE > 0:
                abc_s = consts.tile([H, I_DVE, D], F16)
                nc.gpsimd.dma_start(out=abc_s[:], in_=abc[:])
            dw1a_s = consts.tile([H, H], F32)
            nc.gpsimd.dma_start(out=dw1a_s[:], in_=dw1a[:])
            wf_s = consts.tile([H, H], F32)
            nc.gpsimd.dma_start(out=wf_s[:], in_=wf[:])
            cvec_s = consts.tile([H, 1], F32)
            nc.gpsimd.dma_start(out=cvec_s[:], in_=cvec[:])
            db1_s = consts.tile([H, 1], F32)
            nc.gpsimd.dma_start(out=db1_s[:], in_=db1[:])
            dw2_s = consts.tile([H, 1], F32)
            nc.gpsimd.dma_start(out=dw2_s[:], in_=dw2[:])
            db2_s = consts.tile([1, 1], F32)
            nc.gpsimd.dma_start(out=db2_s[:], in_=db2[:])
            # partition-broadcast loads
            xb_s = consts.tile([H, COLS], F32)
            nc.gpsimd.dma_start(out=xb_s[:], in_=xf[:].to_broadcast([H, COLS]))
            sbc_s = consts.tile([H, COLS], F32)
            nc.gpsimd.dma_start(out=sbc_s[:], in_=stile[:].to_broadcast([H, COLS]))

            def body():
                _body(nc, tc, consts, persist, work, psum, psum_r, psum_z,
                      encw_s, encb_s, w1a_s, w1b_s, b1_s, acol2_s, abc_s,
                      dw1a_s, wf_s, cvec_s, db1_s, dw2_s, db2_s, xb_s, sbc_s,
                      out)

            if repeat == 1:
                body()
            else:
                with tc.For_i(0, repeat, 1):
                    body()

    nc.finalize()
    return nc


def _body(nc, tc, consts, persist, work, psum, psum_r, psum_z,
          encw_s, encb_s, w1a_s, w1b_s, b1_s, acol2_s, abc_s,
          dw1a_s, wf_s, cvec_s, db1_s, dw2_s, db2_s, xb_s, sbc_s, out):
            # --- h^T[m, (b,n)] = tanh(encw[m]*X[b,n] + encb[m]) ---
            hpre = persist.tile([H, COLS], F32)
            nc.vector.tensor_scalar_mul(out=hpre[:], in0=xb_s[:], scalar1=encw_s[:])
            hT = persist.tile([H, COLS], F32)
            nc.scalar.activation(out=hT[:], in_=hpre[:],
                                 func=mybir.ActivationFunctionType.Tanh,
                                 bias=encb_s[:], scale=1.0)

            # --- u^T, v^T via PE;  v gets msg_b1 folded into the relu later ---
            pu = psum.tile([H, COLS], F32, tag="puv")
            for c in range(2):
                nc.tensor.matmul(pu[:, c * 512:(c + 1) * 512], w1a_s[:],
                                 hT[:, c * 512:(c + 1) * 512],
                                 start=True, stop=True)
            uT = persist.tile([H, COLS], F32)
            nc.scalar.activation(out=uT[:], in_=pu[:],
                                 func=mybir.ActivationFunctionType.Copy)

            pv = psum.tile([H, COLS], F32, tag="puv")
            for c in range(2):
                nc.tensor.matmul(pv[:, c * 512:(c + 1) * 512], w1b_s[:],
                                 hT[:, c * 512:(c + 1) * 512],
                                 start=True, stop=True)
            # vT gets msg_b1 folded in, so pre = u + vT already includes it
            vT = persist.tile([H, COLS], F32)
            nc.vector.tensor_scalar_add(out=vT[:], in0=pv[:], scalar1=b1_s[:])

            # --- hot loop: R^T[k, (b,i)] = sum_j A[j,i]*relu(u+v+b1) ---
            # Division of labor per batch:
            #   DVE: one broadcasted add  pre[k,(i,j)] = u[k,j] + v[k,i]  (fp32
            #        reads at 1x, fp16 out)
            #   ACT: relu on the full tile (fp16)
            #   DMA xbar: transpose the 32 (128,128) chunks so (i,j) lands on
            #        partitions
            #   PE:  per chunk, lhsT = transposed chunk, rhs = 2 columns of the
            #        block-masked A (acol2) -> contraction over j in fp32 PSUM.
            # This removes the A-multiply, fold tree, and reduce from DVE
            # entirely; A rides in the PE rhs.
            rall = persist.tile([H, COLS], F32)
            idv = I_DVE                  # i-rows on the DVE reduce path
            npe = D - idv                # i-rows on the transpose+PE path
            nch = (npe * D) // H         # 128-col transpose chunks
            sa = min(S_ADD, D)
            relc = None
            if ABLATE == "onlyreduce":
                relc = persist.tile([H, D, D], F16)
                nc.gpsimd.memset(relc[:], 0.25)
            for b in range(BL):
                if ABLATE == "onlyreduce":
                    rel = relc
                else:
                    ub = uT[:, b * D:(b + 1) * D].unsqueeze(1).to_broadcast(
                        [H, sa, D])
                    vb = vT[:, b * D:b * D + sa].unsqueeze(2).to_broadcast(
                        [H, sa, D])
                    pre = work.tile([H, sa, D], F16)
                    nc.vector.tensor_add(out=pre[:], in0=vb, in1=ub)
                    rel = work.tile([H, D, D], F16)
                    nc.scalar.activation(out=rel[:, 0:sa, :], in_=pre[:],
                                         func=mybir.ActivationFunctionType.Relu)
                    for i in range(sa, D):
                        nc.scalar.activation(
                            out=rel[:, i, :], in_=uT[:, b * D:(b + 1) * D],
                            func=mybir.ActivationFunctionType.Relu,
                            bias=vT[:, b * D + i:b * D + i + 1], scale=1.0)
                if ABLATE == "noreduce":
                    # stand-in for the reduce legs: trivial copy so rall is
                    # still produced per batch (wrong values, same deps)
                    nc.vector.tensor_copy(out=rall[:, b * D:(b + 1) * D],
                                          in_=rel[:, 0, :])
                    continue
                if idv > 0:
                    # DVE path: multiply by A then fold-tree reduce over j
                    prod = work.tile([H, idv, D], F16)
                    nc.vector.tensor_mul(out=prod[:], in0=rel[:, :idv, :],
                                         in1=abc_s[:])
                    f1 = work.tile([H, idv, 32], F16)
                    nc.vector.tensor_add(out=f1[:], in0=prod[:, :, 0:32],
                                         in1=prod[:, :, 32:64])
                    f2 = work.tile([H, idv, 16], F16)
                    nc.vector.tensor_add(out=f2[:], in0=f1[:, :, 0:16],
                                         in1=f1[:, :, 16:32])
                    f3 = work.tile([H, idv, 8], F16)
                    nc.vector.tensor_add(out=f3[:], in0=f2[:, :, 0:8],
                                         in1=f2[:, :, 8:16])
                    nc.vector.tensor_reduce(out=rall[:, b * D:b * D + idv],
                                            in_=f3[:],
                                            axis=mybir.AxisListType.X,
                                            op=mybir.AluOpType.add)
                if npe > 0:
                    # PE path: one xbar transpose for the remaining rows,
                    #   relT[p, c, k] = rel[k, idv*64 + 128c + p]
                    relT = work.tile([H, nch, H], F16)
                    nc.sync.dma_start_transpose(
                        out=relT[:],
                        in_=rel[:, idv:, :].rearrange("p a b -> p (a b)"))
                    rps = psum_r.tile([H, npe], F32)
                    for c in range(nch):
                        nc.tensor.matmul(rps[:, 2 * c:2 * c + 2],
                                         relT[:, c, :],
                                         acol2_s[:, idv + 2 * c:idv + 2 * c + 2],
                                         start=True, stop=True)
                    nc.scalar.activation(out=rall[:, b * D + idv:(b + 1) * D],
                                         in_=rps[:],
                                         func=mybir.ActivationFunctionType.Copy)

            # --- decoder: zpre = dw1a^T-term + Wf^T-term (+ s*cvec) ---
            pz = psum_z.tile([H, COLS], F32, tag="pz")
            for c in range(2):
                sl = slice(c * 512, (c + 1) * 512)
                nc.tensor.matmul(pz[:, sl], dw1a_s[:], hT[:, sl],
                                 start=True, stop=False)
                nc.tensor.matmul(pz[:, sl], wf_s[:], rall[:, sl],
                                 start=False, stop=True)
            sc = persist.tile([H, COLS], F32)
            nc.vector.tensor_scalar_mul(out=sc[:], in0=sbc_s[:], scalar1=cvec_s[:])
            zpre = persist.tile([H, COLS], F32)
            nc.vector.tensor_add(out=zpre[:], in0=pz[:], in1=sc[:])
            zT = persist.tile([H, COLS], F32)
            nc.scalar.activation(out=zT[:], in_=zpre[:],
                                 func=mybir.ActivationFunctionType.Relu,
                                 bias=db1_s[:], scale=1.0)

            po = psum.tile([1, COLS], F32, tag="puv")
            for c in range(2):
                sl = slice(c * 512, (c + 1) * 512)
                nc.tensor.matmul(po[:, sl], dw2_s[:], zT[:, sl],
                                 start=True, stop=True)
            out_s = persist.tile([1, COLS], F32)
            nc.vector.tensor_scalar_add(out=out_s[:], in0=po[:], scalar1=db2_s[:])
            nc.gpsimd.dma_start(out=out[:], in_=out_s[:])


def _get_nc():
    if "nc" not in _CACHE:
        _CACHE["nc"] = _build_bass()
    return _CACHE["nc"]


def make_in_maps(X, W, enc_w, enc_b, msg_w1, msg_b1, msg_w2, msg_b2,
                 dec_w1, dec_b1, dec_w2, dec_b2):
    X = np.asarray(X, np.float32)
    W = np.asarray(W, np.float32)
    enc_w = np.asarray(enc_w, np.float32)
    enc_b = np.asarray(enc_b, np.float32)
    msg_w1 = np.asarray(msg_w1, np.float32)
    msg_b1 = np.asarray(msg_b1, np.float32)
    msg_w2 = np.asarray(msg_w2, np.float32)
    msg_b2 = np.asarray(msg_b2, np.float32)
    dec_w1 = np.asarray(dec_w1, np.float32)
    dec_b1 = np.asarray(dec_b1, np.float32)
    dec_w2 = np.asarray(dec_w2, np.float32)
    dec_b2 = np.asarray(dec_b2, np.float32)

    # host-side weight folds (O(weights) only)
    A = W * (1.0 - np.eye(D, dtype=np.float32))       # masked adjacency
    s = A.sum(axis=0)                                  # (D,)
    # acol2[(i_loc*64 + j), i] = A[j, i] when i % 2 == i_loc else 0 — the
    # block-masked rhs for the per-chunk PE contraction over j.
    acol2 = np.zeros((H, D), np.float16)
    for i in range(D):
        half = (i % 2) * D
        acol2[half:half + D, i] = A[:, i].astype(np.float16)
    assert I_DVE % 2 == 0, "I_DVE must be even (transpose chunks hold i-pairs)"
    wf = (msg_w2 @ dec_w1[H:]).astype(np.float32)      # (H, H)
    cvec = (dec_w1[H:].T @ msg_b2).astype(np.float32)  # (H,)
    stile = np.tile(s, BL).reshape(1, COLS)            # (1, 16*64)

    common = {
        "encw": enc_w.reshape(H, 1),
        "encb": enc_b.reshape(H, 1),
        "w1a": msg_w1[:H].copy(),
        "w1b": msg_w1[H:].copy(),
        "b1": msg_b1.reshape(H, 1),
        "acol2": acol2,
        **({"abc": np.broadcast_to(
                A.T[:I_DVE].astype(np.float16).reshape(1, I_DVE, D),
                (H, I_DVE, D)).copy()} if I_DVE > 0 else {}),
        "dw1a": dec_w1[:H].copy(),
        "wf": wf,
        "cvec": cvec.reshape(H, 1),
        "stile": stile.astype(np.float32),
        "db1": dec_b1.reshape(H, 1),
        "dw2": dec_w2.reshape(H, 1),
        "db2": dec_b2.reshape(1, 1),
    }
    in_maps = []
    for c in range(NCORES):
        m = dict(common)
        m["xf"] = X[c * BL:(c + 1) * BL].reshape(1, COLS).copy()
        in_maps.append(m)
    return in_maps


def kernel(**inputs):
    in_maps = make_in_maps(**inputs)
    nc = _get_nc()
    trace = bool(int(os.environ.get("BASS_KERNEL_TRACE", "0")))
    res = run_bass_kernel_spmd(nc, in_maps, list(range(NCORES)), trace=trace)
    if trace:
        _CACHE["last_result"] = res
    out = np.concatenate(
        [res.results[c]["out"].reshape(BL, D) for c in range(NCORES)], axis=0
    )
    return out.astype(np.float32)
